# revision 1
# baseline (speedup 1.0000x reference)
"""BitLinear (ternary-weight linear + global activation requant) on 8 TRN2 cores.

Computation (see reference):
    wq  = ternarize(weight * scale, thr = 0.7*mean|weight*scale|)   # {-1,0,+1}
    out = x @ wq.T + bias
    s   = 255 / (max(out) - min(out));  out = round(out*s)/s

Sharding: 2x4 grid over (tokens, out_features).  Each core computes a
[4096 tok, 1024 out] shard contracting over the full K=4096.

Speed strategy vs the f32-staging baseline:
  * Matmuls run mixed-precision: bf16 x stationary x fp8 ternary weights
    moving (measured same PE rate as bf16/bf16, exact for ternary values,
    half the weight SBUF).
  * Pass-1 (abs-mean) reads a bf16 copy of only K/2 rows per core (each W
    element counted exactly once across the 8 cores before the AllReduce);
    f32 fidelity is only needed for the pass-2 threshold compare.
  * Ternarize is clip(round(w*m), -1, 1) with m = scale/(2t) via the
    +1.5*2^7 bf16 round-to-even trick: Act (affine+round) -> DVE (sub+min)
    -> DVE (max, fp8 cast), pipelined under the pass-2 weight DMA and the
    first matmul block.
  * Raw outputs stay resident in SBUF as f16 (no DRAM staging round-trip);
    the requant tail ((y*s rounded via +1.5*2^10 f16 trick) * 1/s) is split
    across DVE and Act and writes f16 output (host upcasts to f32).
"""

import numpy as np
import ml_dtypes

import concourse.bass as bass
import concourse.mybir as mybir
import concourse.tile as tile
from concourse.tile import add_dep_helper
from concourse import bacc
from concourse import bass_utils

F32 = mybir.dt.float32
F16 = mybir.dt.float16
BF16 = mybir.dt.bfloat16
FP8 = mybir.dt.float8e4

# Full problem shape
B, S, D_IN, D_OUT = 4, 2048, 4096, 4096
N_CORES = 8
GRID_R, GRID_C = 2, 4  # token shards x out-feature shards
TOK_BLOCK = 256

ACT_FUNC = mybir.ActivationFunctionType

# bf16 round-to-nearest-even magic constant (valid for |y| < 64)
RND_B = 192.0          # 1.5 * 2^7
# f16 round-to-nearest-even magic constant (valid for |y| < 512)
RND_H = 1536.0         # 1.5 * 2^10


def build_kernel(
    tok_per_core: int,
    k_dim: int,
    out_per_core: int,
    tok_block: int,
    n_weight_copies: int = 1,
    debug: bool = False,
    repeat: int = 1,
    use_collectives: bool = True,
    tail_mode: str = "small",  # chunk/ipsmall disabled (HW race)
):
    KO = k_dim // 128
    SUBS = tok_block // 128
    OGW = min(512, out_per_core)
    OGS = out_per_core // OGW
    N_BLOCKS = tok_per_core // tok_block
    K1 = k_dim // GRID_R      # pass-1 rows per core
    K1O = K1 // 128
    n_drains = N_BLOCKS * SUBS * OGS
    assert KO * 128 == k_dim and SUBS * 128 == tok_block
    assert OGS * OGW == out_per_core and N_BLOCKS * tok_block == tok_per_core

    nc = bacc.Bacc(
        "TRN2",
        target_bir_lowering=False,
        debug=debug,
        enable_asserts=False,
        num_devices=N_CORES,
    )

    xt = nc.declare_dram_parameter(
        "xt", [N_BLOCKS, 128, KO, tok_block], BF16, isOutput=False
    )
    wt = nc.declare_dram_parameter("wt", [k_dim, out_per_core], F32, isOutput=False)
    w1 = nc.declare_dram_parameter("w1", [K1O, 128, out_per_core], BF16,
                                   isOutput=False)
    biasv = nc.declare_dram_parameter("biasv", [out_per_core], F32, isOutput=False)
    scalev = nc.declare_dram_parameter("scalev", [1], F32, isOutput=False)
    out = nc.declare_dram_parameter("outv", [tok_per_core, out_per_core], F16,
                                    isOutput=True)

    xt_ap = xt.ap()
    wt_ap = wt.ap()
    w1_ap = w1.ap()
    out_ap = out.ap()

    with tile.TileContext(nc) as tc:
        with (
            tc.tile_pool(name="const", bufs=1) as const_pool,
            tc.tile_pool(name="wslab", bufs=3) as wslab_pool,
            tc.tile_pool(name="tern", bufs=2) as tern_pool,  # y/t1 bf16
            tc.tile_pool(name="wq", bufs=1) as wq_pool,
            tc.tile_pool(name="xbuf", bufs=2) as x_pool,
            tc.tile_pool(name="drain", bufs=1) as drain_pool,
            tc.tile_pool(name="qt", bufs=2) as q_pool,
            tc.tile_pool(name="psum", bufs=2, space="PSUM") as psum_pool,
            tc.tile_pool(name="dram", bufs=1, space="DRAM") as dram_pool,
        ):

            def accurate_recip(out_ap2, in_ap, tag):
                # r1 = r0*(2 - x*r0), one Newton step on InstReciprocal
                r0 = const_pool.tile([1, 1], F32, tag=f"{tag}_r0")
                nc.vector.reciprocal(r0, in_ap)
                e = const_pool.tile([1, 1], F32, tag=f"{tag}_e")
                nc.vector.tensor_scalar(e, in_ap, r0, None, mybir.AluOpType.mult)
                nc.vector.tensor_scalar(
                    e, e, -1.0, 2.0, mybir.AluOpType.mult, mybir.AluOpType.add
                )
                nc.vector.tensor_mul(out_ap2, r0, e)

            def phase_consts():
                scale_sb = const_pool.tile([1, 1], F32, tag="scale_sb")
                nc.sync.dma_start(scale_sb, scalev.ap()[None, :])
                bias_sb = const_pool.tile([1, out_per_core], F32, tag="bias_sb")
                nc.sync.dma_start(bias_sb, biasv.ap()[None, :])
                bias_b = const_pool.tile([128, out_per_core], F32, tag="bias_b")
                nc.gpsimd.partition_broadcast(bias_b, bias_sb)
                return scale_sb, bias_b

            def phase_w_stats(scale_sb, defer_insts=()):
                """Global S = sum|w| (each W element counted once across cores)
                -> m = scale / (2 * 0.7 * mean|w*scale|), broadcast to m_b."""
                G = 4  # pass-1 slabs per DMA/reduce group
                n_g = K1O // G
                wsum = const_pool.tile([128, n_g], F32, tag="wsum")
                p1_gate = None
                for g in range(n_g):
                    wb = wslab_pool.tile([128, G, out_per_core], BF16,
                                         tag="w1slab", bufs=2)
                    p1_gate = nc.sync.dma_start(
                        wb, w1_ap[g * G:(g + 1) * G].rearrange("g p o -> p g o")
                    )
                    nc.vector.tensor_reduce(
                        wsum[:, g:g + 1], wb,
                        axis=mybir.AxisListType.XY,
                        op=mybir.AluOpType.add, apply_absolute_value=True,
                    )
                # keep pass-1 at full DMA bandwidth: deferred prefetches start
                # only once its last slab is issued
                for di in defer_insts:
                    add_dep_helper(di.ins, p1_gate.ins, sync=True,
                                   reason="defer prefetch behind pass-1 W DMA")

                wsum1 = const_pool.tile([128, 1], F32, tag="wsum1")
                nc.vector.tensor_reduce(
                    wsum1, wsum, axis=mybir.AxisListType.X, op=mybir.AluOpType.add
                )
                wsum_all = const_pool.tile([128, 1], F32, tag="wsum_all")
                nc.gpsimd.partition_all_reduce(
                    wsum_all, wsum1, 128, bass.bass_isa.ReduceOp.add
                )

                cc1_in = dram_pool.tile([1, 1], F32, tag="cc1_in")
                cc1_out = dram_pool.tile([1, 1], F32, tag="cc1_out")
                nc.sync.dma_start(cc1_in, wsum_all[0:1, :])
                if use_collectives:
                    nc.gpsimd.collective_compute(
                        "AllReduce",
                        mybir.AluOpType.add,
                        replica_groups=[list(range(N_CORES))],
                        ins=[cc1_in.opt()],
                        outs=[cc1_out.opt()],
                    )
                else:
                    nc.sync.dma_start(cc1_out, cc1_in)
                s_glob = const_pool.tile([1, 1], F32, tag="s_glob")
                nc.sync.dma_start(s_glob, cc1_out)

                # m = sign(scale) * N / (1.4*S) = scale * N/1.4 * 1/(|scale|*S)
                n_w_elems = float(k_dim * GRID_C * out_per_core)
                tcoef = float(np.float32(n_w_elems) / np.float32(1.4))
                absscale = const_pool.tile([1, 1], F32, tag="absscale")
                nc.vector.tensor_reduce(
                    absscale, scale_sb, axis=mybir.AxisListType.X,
                    op=mybir.AluOpType.max, apply_absolute_value=True,
                )
                den = const_pool.tile([1, 1], F32, tag="den")
                nc.vector.tensor_scalar(
                    den, s_glob, absscale, None, mybir.AluOpType.mult
                )
                rden = const_pool.tile([1, 1], F32, tag="rden")
                accurate_recip(rden, den, "rd")
                m = const_pool.tile([1, 1], F32, tag="m")
                nc.vector.tensor_scalar(m, scale_sb, rden, None, mybir.AluOpType.mult)
                nc.vector.tensor_scalar(m, m, tcoef, None, mybir.AluOpType.mult)
                m_b = const_pool.tile([128, 1], F32, tag="m_b")
                nc.gpsimd.partition_broadcast(m_b, m)
                return m_b, p1_gate

            def phase_w_tern(m_b, p1_gate=None, after_slab2=None):
                """wq = clip(round(w*m), -1, 1) -> fp8, slab-pipelined over
                Act (affine+round) -> DVE (sub+min) -> DVE (max, fp8 cast)."""
                wq = wq_pool.tile([128, KO, out_per_core], FP8, tag="wq")
                for ko in range(KO):
                    if ko == 16 and after_slab2 is not None:
                        after_slab2()
                    wsl = wslab_pool.tile([128, out_per_core], F32, tag="wslab")
                    d2 = nc.sync.dma_start(wsl, wt_ap[ko * 128:(ko + 1) * 128, :])
                    if p1_gate is not None:
                        add_dep_helper(d2.ins, p1_gate.ins, sync=True,
                                       reason="pass-2 W after pass-1 issued")
                    y = tern_pool.tile([128, out_per_core], BF16, tag="tern_y",
                                       bufs=2)
                    nc.scalar.activation(y, wsl, ACT_FUNC.Copy, bias=RND_B,
                                         scale=m_b)
                    t1 = tern_pool.tile([128, out_per_core], BF16, tag="tern_t1",
                                        bufs=2)
                    nc.vector.tensor_scalar(
                        t1, y, RND_B, 1.0,
                        mybir.AluOpType.subtract, mybir.AluOpType.min,
                    )
                    nc.vector.tensor_scalar(
                        wq[:, ko, :], t1, -1.0, None, mybir.AluOpType.max
                    )
                return wq

            def phase_mm(wq, bias_b, xpref=None):
                """Matmul blocks (bf16 x stationary, fp8 wq moving): accumulate
                K in PSUM, +bias -> f16 SBUF-resident drains, max/min stats."""
                maxst = const_pool.tile([128, n_drains], F32, tag="maxst")
                minst = const_pool.tile([128, n_drains], F32, tag="minst")
                drains_t = [
                    drain_pool.tile([128, OGS * OGW], F16, name=f"drow_{i}")
                    for i in range(N_BLOCKS * SUBS)
                ]

                for blk in range(N_BLOCKS):
                    if xpref and blk in xpref:
                        x_tile = xpref.pop(blk)
                    else:
                        x_tile = x_pool.tile([128, KO, tok_block], BF16,
                                             tag="x_tile")
                        nc.sync.dma_start(x_tile, xt_ap[blk])
                    psums = [
                        [
                            psum_pool.tile([128, OGW], F32, name=f"ps_{sub}_{og}")
                            for og in range(OGS)
                        ]
                        for sub in range(SUBS)
                    ]
                    for ko in range(KO):
                        for sub in range(SUBS):
                            lhsT = x_tile[:, ko, sub * 128:(sub + 1) * 128]
                            for og in range(OGS):
                                nc.tensor.matmul(
                                    psums[sub][og],
                                    lhsT,
                                    wq[:, ko, og * OGW:(og + 1) * OGW],
                                    start=(ko == 0),
                                    stop=(ko == KO - 1),
                                )
                    for sub in range(SUBS):
                        bs = blk * SUBS + sub
                        for og in range(OGS):
                            idx = bs * OGS + og
                            d = drains_t[bs][:, og * OGW:(og + 1) * OGW]
                            nc.vector.tensor_add(
                                d, psums[sub][og],
                                bias_b[:, og * OGW:(og + 1) * OGW],
                            )
                            nc.vector.tensor_reduce(
                                maxst[:, idx:idx + 1], d,
                                axis=mybir.AxisListType.X, op=mybir.AluOpType.max,
                            )
                            nc.vector.tensor_reduce(
                                minst[:, idx:idx + 1], d,
                                axis=mybir.AxisListType.X, op=mybir.AluOpType.min,
                            )
                return maxst, minst, drains_t

            def phase_tail(maxst, minst, drains):
                """Global max/min -> s -> requantize resident drains -> out."""
                lmax = const_pool.tile([128, 1], F32, tag="lmax")
                lmin = const_pool.tile([128, 1], F32, tag="lmin")
                nc.vector.tensor_reduce(
                    lmax, maxst, axis=mybir.AxisListType.X, op=mybir.AluOpType.max
                )
                nc.vector.tensor_reduce(
                    lmin, minst, axis=mybir.AxisListType.X, op=mybir.AluOpType.min
                )
                st2 = const_pool.tile([128, 2], F32, tag="st2")
                nc.vector.tensor_copy(out=st2[:, 0:1], in_=lmax)
                nc.vector.tensor_scalar_mul(st2[:, 1:2], lmin, -1.0)
                st2r = const_pool.tile([128, 2], F32, tag="st2r")
                nc.gpsimd.partition_all_reduce(
                    st2r, st2, 128, bass.bass_isa.ReduceOp.max
                )

                cc2_in = dram_pool.tile([1, 2], F32, tag="cc2_in")
                cc2_out = dram_pool.tile([1, 2], F32, tag="cc2_out")
                nc.sync.dma_start(cc2_in, st2r[0:1, :])
                if use_collectives:
                    nc.gpsimd.collective_compute(
                        "AllReduce",
                        mybir.AluOpType.max,
                        replica_groups=[list(range(N_CORES))],
                        ins=[cc2_in.opt()],
                        outs=[cc2_out.opt()],
                    )
                else:
                    nc.sync.dma_start(cc2_out, cc2_in)
                gst = const_pool.tile([1, 2], F32, tag="gst")
                nc.sync.dma_start(gst, cc2_out)

                rng = const_pool.tile([1, 1], F32, tag="rng")  # max - min
                nc.vector.tensor_reduce(
                    rng, gst, axis=mybir.AxisListType.X, op=mybir.AluOpType.add
                )

                sq = const_pool.tile([1, 2], F32, tag="sq")  # [s, 1/s]
                rinv = const_pool.tile([1, 1], F32, tag="rinv")
                accurate_recip(rinv, rng, "rr")
                nc.vector.tensor_scalar_mul(sq[:, 0:1], rinv, 255.0)
                accurate_recip(sq[:, 1:2], sq[:, 0:1], "si")
                b2 = const_pool.tile([1, 1], F32, tag="b2")  # -RND_H / s
                nc.vector.tensor_scalar_mul(b2, sq[:, 1:2], -RND_H)
                sq_b = const_pool.tile([128, 2], F32, tag="sq_b")
                nc.gpsimd.partition_broadcast(sq_b, sq)
                b2_b = const_pool.tile([128, 1], F32, tag="b2_b")
                nc.gpsimd.partition_broadcast(b2_b, b2)

                # q = round(y*s)/s via f16 RNE with +/- 1536: pass 1 into a
                # scratch tile, pass 2 back into the drain buffer in place;
                # ~5:3 tiles DVE:Act. Output DMA goes out in 4 big chunks.
                drains_t = drains
                BS = N_BLOCKS * SUBS
                W2 = OGS * OGW
                out_r = out_ap.rearrange("(bs p) oc -> p bs oc", p=128)
                if False:
                    CH = BS // 4
                    for c in range(4):
                        for bs in range(c * CH, (c + 1) * CH):
                            d = drains_t[:, bs, :]
                            q1 = q_pool.tile([128, W2], F16, tag="q1", bufs=4)
                            if bs % 8 < 5:
                                nc.vector.tensor_scalar(
                                    q1, d, sq_b[:, 0:1], RND_H,
                                    mybir.AluOpType.mult, mybir.AluOpType.add,
                                )
                                nc.vector.tensor_scalar(
                                    d, q1, RND_H, sq_b[:, 1:2],
                                    mybir.AluOpType.subtract,
                                    mybir.AluOpType.mult,
                                )
                            else:
                                nc.scalar.activation(q1, d, ACT_FUNC.Copy,
                                                     bias=RND_H,
                                                     scale=sq_b[:, 0:1])
                                nc.scalar.activation(d, q1, ACT_FUNC.Identity,
                                                     bias=b2_b,
                                                     scale=sq_b[:, 1:2])
                        nc.sync.dma_start(
                            out_r[:, c * CH:(c + 1) * CH, :],
                            drains_t[:, c * CH:(c + 1) * CH, :],
                        )
                elif False:
                    for bs in range(BS):
                        d = drains_t[:, bs, :]
                        q1 = q_pool.tile([128, W2], F16, tag="q1", bufs=4)
                        if bs % 8 < 5:
                            nc.vector.tensor_scalar(
                                q1, d, sq_b[:, 0:1], RND_H,
                                mybir.AluOpType.mult, mybir.AluOpType.add,
                            )
                            nc.vector.tensor_scalar(
                                d, q1, RND_H, sq_b[:, 1:2],
                                mybir.AluOpType.subtract, mybir.AluOpType.mult,
                            )
                        else:
                            nc.scalar.activation(q1, d, ACT_FUNC.Copy,
                                                 bias=RND_H,
                                                 scale=sq_b[:, 0:1])
                            nc.scalar.activation(d, q1, ACT_FUNC.Identity,
                                                 bias=b2_b,
                                                 scale=sq_b[:, 1:2])
                        nc.sync.dma_start(out_r[:, bs, :], d)
                else:  # per-bs-row requant + small DMA, no in-place
                    for bs in range(BS):
                        d = drains_t[bs]
                        q1 = q_pool.tile([128, W2], F16, tag="q1", bufs=4)
                        q2 = q_pool.tile([128, W2], F16, tag="q2", bufs=4)
                        if bs % 8 < 5:
                            nc.vector.tensor_scalar(
                                q1, d, sq_b[:, 0:1], RND_H,
                                mybir.AluOpType.mult, mybir.AluOpType.add,
                            )
                            nc.vector.tensor_scalar(
                                q2, q1, RND_H, sq_b[:, 1:2],
                                mybir.AluOpType.subtract, mybir.AluOpType.mult,
                            )
                        else:
                            nc.scalar.activation(q1, d, ACT_FUNC.Copy,
                                                 bias=RND_H,
                                                 scale=sq_b[:, 0:1])
                            nc.scalar.activation(q2, q1, ACT_FUNC.Identity,
                                                 bias=b2_b,
                                                 scale=sq_b[:, 1:2])
                        nc.sync.dma_start(out_r[:, bs, :], q2)

            for _ in range(repeat):
                # prefetch x block 0 behind the pass-1 W DMA; x block 1 is
                # issued mid-ternarize so pass-2 slabs reach the queue first
                xpref = {}
                xp0 = x_pool.tile([128, KO, tok_block], BF16, tag="x_tile")
                xi0 = nc.sync.dma_start(xp0, xt_ap[0])
                xpref[0] = xp0

                def prefetch_x1():
                    xp1 = x_pool.tile([128, KO, tok_block], BF16, tag="x_tile")
                    nc.sync.dma_start(xp1, xt_ap[1])
                    xpref[1] = xp1

                scale_sb, bias_b = phase_consts()
                m_b, p1_gate = phase_w_stats(scale_sb, [xi0])
                wq = phase_w_tern(m_b, p1_gate, after_slab2=prefetch_x1)
                maxst, minst, drains = phase_mm(wq, bias_b, xpref)
                phase_tail(maxst, minst, drains)

    nc.compile()
    return nc


_NC_CACHE: dict = {}


def _get_full_nc():
    key = "full"
    if key not in _NC_CACHE:
        _NC_CACHE[key] = build_kernel(
            tok_per_core=(B * S) // GRID_R,
            k_dim=D_IN,
            out_per_core=D_OUT // GRID_C,
            tok_block=TOK_BLOCK,
            n_weight_copies=GRID_R,
            debug=False,
        )
    return _NC_CACHE[key]


def make_in_maps(x, weight, bias, scale, grid_r=GRID_R, grid_c=GRID_C,
                 tok_block=TOK_BLOCK):
    """Host-side layout prep: transpose/cast/shard. No arithmetic on values."""
    x = np.asarray(x, dtype=np.float32)
    weight = np.asarray(weight, dtype=np.float32)
    bias = np.asarray(bias, dtype=np.float32)
    scale = np.asarray(scale, dtype=np.float32)

    n_tok = x.size // x.shape[-1]
    k_dim = x.shape[-1]
    d_out = weight.shape[0]
    tok_pc = n_tok // grid_r
    out_pc = d_out // grid_c
    n_blocks = tok_pc // tok_block
    k1 = k_dim // grid_r

    xf = x.reshape(n_tok, k_dim)
    # [k, n_tok] bf16 (single transpose+cast pass)
    xtb = xf.T.astype(ml_dtypes.bfloat16)
    wt_full = np.ascontiguousarray(weight.T)  # [k, d_out]

    in_maps = []
    for cid in range(grid_r * grid_c):
        r, c = divmod(cid, grid_c)
        xs = xtb[:, r * tok_pc:(r + 1) * tok_pc]  # [k, tok_pc]
        # -> [n_blocks, 128, ko, tok_block] (partition-major: k = ko*128 + p)
        xs = np.ascontiguousarray(
            xs.reshape(k_dim // 128, 128, n_blocks, tok_block)
            .transpose(2, 1, 0, 3)
        )
        wshard = wt_full[:, c * out_pc:(c + 1) * out_pc]
        in_maps.append(
            {
                "xt": xs,
                "wt": np.ascontiguousarray(wshard),
                "w1": np.ascontiguousarray(
                    wshard[r * k1:(r + 1) * k1, :]
                ).astype(ml_dtypes.bfloat16).reshape(k1 // 128, 128, out_pc),
                "biasv": np.ascontiguousarray(bias[c * out_pc:(c + 1) * out_pc]),
                "scalev": scale.reshape(1),
            }
        )
    return in_maps


def assemble_out(results, out_shape, grid_r=GRID_R, grid_c=GRID_C):
    n_tok = int(np.prod(out_shape[:-1]))
    d_out = out_shape[-1]
    tok_pc = n_tok // grid_r
    out_pc = d_out // grid_c
    full = np.empty((n_tok, d_out), dtype=np.float32)
    for cid in range(grid_r * grid_c):
        r, c = divmod(cid, grid_c)
        full[r * tok_pc:(r + 1) * tok_pc, c * out_pc:(c + 1) * out_pc] = (
            results[cid]["outv"].astype(np.float32)
        )
    return full.reshape(out_shape)


def kernel(x, weight, bias, scale):
    nc = _get_full_nc()
    in_maps = make_in_maps(x, weight, bias, scale)
    res = bass_utils.run_bass_kernel_spmd(nc, in_maps, core_ids=list(range(N_CORES)))
    return assemble_out(res.results, (B, S, D_OUT))

